# revision 5
# baseline (speedup 1.0000x reference)
"""HDTimeCrystalBlock kernel for 8 Trainium2 NeuronCores.

Math: out = ((x @ W_in) * mod[None]) @ W_out, where
  mod[l,h] = sum_m coupled[m] * cos(omega*(m+1)*t[l] + E[m,h])
Using cos(a+b) = cos(a)cos(b) - sin(a)sin(b):
  mod = C @ A + S @ B,  A[m,h] = coupled[m]*cos(E[m,h]),
                        B[m,h] = -coupled[m]*sin(E[m,h]),
  C[l,m] = cos(omega*(m+1)*t[l]), S[l,m] = sin(...)
so the [L,M,HD] cos tensor never materializes: mod is a K=2M matmul.

Sharding: data-parallel over the 8192 tokens (B*L), 1024 per core; weights
replicated. Activations stay transposed ([feature, token]) on device so both
matmuls consume natural-layout weights as the stationary operand; the host
does the x/y transposes. Matmuls run in float32r (full PE rate, ~1e-4 rel err).
E's cos/sin are computed on device with sign-based range reduction to [-pi,pi]
(ACT Sin LUT is only accurate there).
"""
import math

import numpy as np

B, L, D, HD, M = 4, 2048, 512, 4096, 16
NCORES = 8
T = (B * L) // NCORES          # tokens per core
QCH = 512                      # l-chunk (PSUM bank width in fp32)
NQ = T // QCH
NJ = HD // 128                 # h-tiles
NK = D // 128                  # d-tiles
PI = math.pi

_cache = {}


def _build():
    from concourse import bacc, bass, mybir, tile

    F32 = mybir.dt.float32
    F32R = mybir.dt.float32r
    AF = mybir.ActivationFunctionType
    PSUM = bass.MemorySpace.PSUM

    nc = bacc.Bacc("TRN2", target_bir_lowering=False, debug=False)

    xT_d = nc.dram_tensor("xT", [D, T], F32R, kind="ExternalInput")
    w_in_d = nc.dram_tensor("w_in", [D, HD], F32R, kind="ExternalInput")
    w_out_d = nc.dram_tensor("w_out", [HD, D], F32R, kind="ExternalInput")
    cs_d = nc.dram_tensor("cs", [2 * M, T], F32R, kind="ExternalInput")
    fe_d = nc.dram_tensor("fe", [128, M * HD // 128], F32, kind="ExternalInput")
    cb_d = nc.dram_tensor("cb", [128, 1], F32, kind="ExternalInput")
    cbn_d = nc.dram_tensor("cbn", [128, 1], F32, kind="ExternalInput")
    yT_d = nc.dram_tensor("yT", [D, T], F32, kind="ExternalOutput")

    FEW = M * HD // 128        # 512

    # Register the activation-bias constants (same mechanism as the 0.0/1.0
    # consts in Bass.__init__): non-Copy activations need an SBUF const AP.
    for val in (-PI, PI / 2):
        t_ = nc.alloc_sbuf_tensor(f"const-float32-{val}", [128, 1], F32)
        nc.gpsimd.memset(t_.ap(), val)
        nc.const_aps.aps[(F32, val)] = t_.ap()
    nc.all_engine_barrier()

    with tile.TileContext(nc) as tc:
        with (
            tc.tile_pool(name="win", bufs=1) as winp,
            tc.tile_pool(name="wout", bufs=1) as woutp,
            tc.tile_pool(name="xts", bufs=1) as xtp,
            tc.tile_pool(name="small", bufs=1) as smallp,
            tc.tile_pool(name="prep", bufs=1) as prepp,
            tc.tile_pool(name="hm", bufs=3) as hmp,
            tc.tile_pool(name="mods", bufs=3) as modsp,
            tc.tile_pool(name="yo", bufs=2) as yop,
            tc.tile_pool(name="pa", bufs=2, space=PSUM) as pap,
            tc.tile_pool(name="pb", bufs=2, space=PSUM) as pbp,
            tc.tile_pool(name="py", bufs=4, space=PSUM) as pyp,
        ):
            # ---- small inputs first (they gate the mod path) ----
            fe = prepp.tile([128, FEW], F32, tag="fe")
            cb = smallp.tile([128, 1], F32, tag="cb")
            cbn = smallp.tile([128, 1], F32, tag="cbn")
            cs = smallp.tile([2 * M, T], F32R, tag="cs")
            nc.sync.dma_start(fe[:], fe_d[:])
            nc.sync.dma_start(cb[:], cb_d[:])
            nc.sync.dma_start(cbn[:], cbn_d[:])
            nc.sync.dma_start(cs[:], cs_d[:])

            # ---- E -> A=(cb*cosE), B=(-cb*sinE), in [128,512] layout ----
            sgn = prepp.tile([128, FEW], F32, tag="sgn")
            wk = prepp.tile([128, FEW], F32, tag="wk")
            er = prepp.tile([128, FEW], F32, tag="er")
            nc.scalar.activation(sgn[:], fe[:], AF.Sign)           # sign(E)
            nc.scalar.activation(wk[:], fe[:], AF.Abs)             # |E|
            nc.scalar.activation(wk[:], wk[:], AF.Sign, bias=-PI)  # sign(|E|-pi)
            nc.scalar.activation(wk[:], wk[:], AF.Copy, bias=PI, scale=PI)
            nc.vector.tensor_mul(wk[:], wk[:], sgn[:])             # {0,±2pi}
            nc.vector.tensor_sub(er[:], fe[:], wk[:])              # Er in [-pi,pi]
            nc.scalar.activation(sgn[:], er[:], AF.Sin)            # sin(E)
            nc.scalar.activation(wk[:], er[:], AF.Abs)             # |Er|
            nc.scalar.activation(er[:], wk[:], AF.Sin, bias=PI / 2, scale=-1.0)
            # er = cos(E), sgn = sin(E)
            nc.vector.tensor_scalar_mul(er[:], er[:], cb[:, 0:1])    # A
            nc.vector.tensor_scalar_mul(sgn[:], sgn[:], cbn[:, 0:1])  # B

            # ---- reshape A,B [128,512] -> ab [2M, HD] (partition m) ----
            ab = smallp.tile([2 * M, HD], F32R, tag="ab")
            for m in range(M):
                nc.gpsimd.dma_start(ab[m : m + 1, :], er[8 * m : 8 * m + 8, :])
                nc.gpsimd.dma_start(ab[M + m : M + m + 1, :], sgn[8 * m : 8 * m + 8, :])

            # ---- bulk weight/activation loads ----
            win = []
            xts = []
            for k in range(NK):
                t_ = winp.tile([128, HD], F32R, tag=f"win{k}")
                nc.sync.dma_start(t_[:], w_in_d[128 * k : 128 * (k + 1), :])
                win.append(t_)
                tx = xtp.tile([128, T], F32R, tag=f"xts{k}")
                nc.sync.dma_start(tx[:], xT_d[128 * k : 128 * (k + 1), :])
                xts.append(tx)
            wout = []
            for j in range(NJ):
                tw = woutp.tile([128, D], F32R, tag=f"wout{j}")
                nc.sync.dma_start(tw[:], w_out_d[128 * j : 128 * (j + 1), :])
                wout.append(tw)

            # ---- fused main loop ----
            for q in range(NQ):
                lo, hi = q * QCH, (q + 1) * QCH
                pys = [pyp.tile([128, QCH], F32, name=f"py{q}_{j2}", tag="py")
                       for j2 in range(NK)]
                for j in range(NJ):
                    pa = pap.tile([128, QCH], F32, tag="pa")
                    for k in range(NK):
                        nc.tensor.matmul(
                            pa[:],
                            win[k][:, 128 * j : 128 * (j + 1)],
                            xts[k][:, lo:hi],
                            start=(k == 0),
                            stop=(k == NK - 1),
                        )
                    pb = pbp.tile([128, QCH], F32, tag="pb")
                    nc.tensor.matmul(
                        pb[:],
                        ab[:, 128 * j : 128 * (j + 1)],
                        cs[:, lo:hi],
                        start=True,
                        stop=True,
                    )
                    msb = modsp.tile([128, QCH], F32, tag="mods")
                    nc.scalar.copy(msb[:], pb[:])
                    hm = hmp.tile([128, QCH], F32R, tag="hm")
                    nc.vector.tensor_mul(hm[:], pa[:], msb[:])
                    for j2 in range(NK):
                        nc.tensor.matmul(
                            pys[j2][:],
                            wout[j][:, 128 * j2 : 128 * (j2 + 1)].bitcast(F32R),
                            hm[:],
                            start=(j == 0),
                            stop=(j == NJ - 1),
                        )
                for j2 in range(NK):
                    yo = yop.tile([128, QCH], F32, tag="yo")
                    nc.scalar.copy(yo[:], pys[j2][:])
                    nc.sync.dma_start(
                        yT_d[128 * j2 : 128 * (j2 + 1), lo:hi], yo[:]
                    )

    nc.finalize()
    return nc


def _get_nc():
    if "nc" not in _cache:
        _cache["nc"] = _build()
    return _cache["nc"]


def _in_maps(x, input_proj, output_proj, floquet_energies, drive_weights,
             coupling_matrix):
    coupled = coupling_matrix.astype(np.float64) @ drive_weights.astype(np.float64)
    cb = np.repeat(coupled, 128 // M).astype(np.float32).reshape(128, 1)
    cbn = (-cb).copy()
    fe = np.ascontiguousarray(
        floquet_energies.astype(np.float32).reshape(128, M * HD // 128)
    )
    w_in = np.ascontiguousarray(input_proj.astype(np.float32))
    w_out = np.ascontiguousarray(output_proj.astype(np.float32))

    harm = np.arange(1, M + 1, dtype=np.float64)
    maps = []
    for c in range(NCORES):
        b, half = c // 2, c % 2
        t = (half * T + np.arange(T, dtype=np.float64)) / L
        ang = 2.0 * np.pi * harm[:, None] * t[None, :]
        cs = np.concatenate([np.cos(ang), np.sin(ang)], axis=0).astype(np.float32)
        xT = np.ascontiguousarray(x[b, half * T : (half + 1) * T, :].T)
        maps.append(
            {
                "xT": xT,
                "w_in": w_in,
                "w_out": w_out,
                "cs": np.ascontiguousarray(cs),
                "fe": fe,
                "cb": cb,
                "cbn": cbn,
            }
        )
    return maps


def kernel(x, input_proj, output_proj, floquet_energies, drive_weights,
           coupling_matrix, _trace=False, _trace_kwargs=None):
    from concourse.bass_utils import run_bass_kernel_spmd

    nc = _get_nc()
    maps = _in_maps(x, input_proj, output_proj, floquet_energies,
                    drive_weights, coupling_matrix)
    kw = dict(_trace_kwargs or {})
    res = run_bass_kernel_spmd(nc, maps, list(range(NCORES)), trace=_trace, **kw)
    out = np.empty((B, L, D), dtype=np.float32)
    for c in range(NCORES):
        b, half = c // 2, c % 2
        out[b, half * T : (half + 1) * T, :] = res.results[c]["yT"].T
    if _trace:
        return out, res
    return out
